# revision 11
# baseline (speedup 1.0000x reference)
"""
CosmosUnpatcher3d (inverse 3D Haar wavelet, PATCH_SIZE=2) on 8 Trainium2
NeuronCores.

Math: input  x[b, ch, i, j, k] with ch = 12*bt + 6*bh + 3*bw + c
      output y[b, c, t, h, w]  with t = 2i+dt, h = 2j+dh, w = 2k+dw
      y = sum_{bt,bh,bw} (-1)^(bt*dt + bh*dh + bw*dw) * x[...]
(the Haar taps (1/sqrt2)^3 times the final sqrt(8) rescale cancel to
exactly 1.0), then the t=0 plane is dropped (17 output t-planes).

Sharding: 8 cores = batch(2) x H-quarters(4); each core gets 64 input
H-rows -> 128 output h-rows.

The host packs each core's shard partition-major so every DMA is a flat
[128, N] stream:
  x_arr[jc, p=(q*4+g2), (i, bwc, k)]   g2=(bt,bh), bwc=(bw*3+c)
Per-core kernel, for each (jc, i-group):
  - DMA in   rhs [128, L*1536]
  - PE       block-diag (I_32 (x) H2 (x) H2) 128x128 fp32 matmul:
             contracts (bt,bh) -> (dt,dh) on partitions,
             psum[(q,dt,dh), (bw,c,k)]
  - ScalarE  copy bw=1 half of PSUM -> SBUF
  - DVE      add/sub (bw=0 PSUM, bw=1 SBUF) writing the w-interleaved
             (w=2k+dw) result z[(q,dt,dh), (i, c, w)]
  - DMA out  z -> out_raw[jc, p, (i, c, w)]
Host unpacks out_raw -> (b, 3, 17, 512, 512).
"""

import numpy as np

_N_CORES = 8
_B, _CH, _TI, _HI, _WI = 2, 24, 9, 256, 256
_C_OUT = 3
_JQ = 4               # H-quarter cores per batch entry
_HJ = _HI // _JQ      # 64 input rows per core
_I_GROUPS = [(0, 4), (4, 4), (8, 1)]

_cached = {}


def _build_hadamard():
    """lhsT[p, m]: p = q*4 + (bt*2+bh), m = q*4 + (dt*2+dh), block-diag."""
    had = np.zeros((128, 128), dtype=np.float32)
    for q in range(32):
        for bt in range(2):
            for bh in range(2):
                for dt in range(2):
                    for dh in range(2):
                        had[q * 4 + bt * 2 + bh, q * 4 + dt * 2 + dh] = (
                            -1.0 if (bt * dt + bh * dh) % 2 else 1.0
                        )
    return had


def _build_nc(repeat=1):
    import concourse.bacc as bacc
    import concourse.mybir as mybir
    from concourse.tile import TileContext
    from concourse.mybir import AluOpType
    from contextlib import ExitStack

    f32 = mybir.dt.float32
    nc = bacc.Bacc()
    X = nc.declare_dram_parameter(
        "x", [2, 128, _TI * 6 * 256], f32, isOutput=False
    )
    H = nc.declare_dram_parameter("had", [128, 128], f32, isOutput=False)
    O = nc.declare_dram_parameter(
        "out", [2, 128, _TI * 3 * 512], f32, isOutput=True
    )

    with TileContext(nc) as tc, ExitStack() as ctx:
        const = ctx.enter_context(tc.tile_pool(name="const", bufs=1))
        rhsp = ctx.enter_context(tc.tile_pool(name="rhs", bufs=2))
        zp = ctx.enter_context(tc.tile_pool(name="z", bufs=2))
        sb1p = ctx.enter_context(tc.tile_pool(name="sb1", bufs=3))
        psp = ctx.enter_context(tc.tile_pool(name="ps", bufs=2, space="PSUM"))

        had_t = const.tile([128, 128], f32)
        nc.sync.dma_start(out=had_t[:], in_=H[:])

        for _rep, jc in [(r, j) for r in range(repeat) for j in range(2)]:
            for (i0, L) in _I_GROUPS:
                rhs = rhsp.tile([128, L * 1536], f32)
                nc.sync.dma_start(
                    out=rhs[:], in_=X[jc, :, i0 * 1536 : (i0 + L) * 1536]
                )
                z = zp.tile([128, L * 1536], f32)
                zv = z[:].rearrange(
                    "p (i cc k dw) -> p i cc dw k", cc=3, dw=2, k=256
                )
                for idx in range(L):
                    ps = psp.tile([128, 1536], f32)
                    base = idx * 1536
                    for m in range(3):
                        nc.tensor.matmul(
                            ps[:, m * 512 : (m + 1) * 512],
                            had_t[:],
                            rhs[:, base + m * 512 : base + (m + 1) * 512],
                            start=True,
                            stop=True,
                        )
                    sb1 = sb1p.tile([128, 768], f32)
                    nc.scalar.copy(sb1[:], ps[:, 768:1536])
                    pv = ps[:, 0:768].rearrange("p (cc k) -> p cc k", cc=3)
                    sv = sb1[:].rearrange("p (cc k) -> p cc k", cc=3)
                    nc.vector.tensor_tensor(
                        zv[:, idx, :, 0, :], pv, sv, AluOpType.add
                    )
                    nc.vector.tensor_tensor(
                        zv[:, idx, :, 1, :], pv, sv, AluOpType.subtract
                    )
                nc.scalar.dma_start(
                    out=O[jc, :, i0 * 1536 : (i0 + L) * 1536], in_=z[:]
                )
    nc.finalize()
    return nc


def _get_nc():
    import os

    rep = int(os.environ.get("K_NC_REPEAT", "1"))
    key = ("nc", rep)
    if key not in _cached:
        _cached[key] = _build_nc(rep)
    return _cached[key]


def _pack_core(xb, jq):
    """xb: (24, 9, 256, 256) full-batch-entry input; returns the core's
    packed (2, 128, 9*6*256) array for H-quarter jq."""
    xs = xb[:, :, jq * _HJ : (jq + 1) * _HJ, :]          # (24, 9, 64, 256)
    v = xs.reshape(4, 6, _TI, 2, 32, 256)                # (g2,bwc,i,jc,q,k)
    v = v.transpose(3, 4, 0, 2, 1, 5)                    # (jc,q,g2,i,bwc,k)
    return np.ascontiguousarray(v).reshape(2, 128, _TI * 6 * 256)


def kernel(hidden_states: np.ndarray) -> np.ndarray:
    import os
    from concourse.bass_utils import run_bass_kernel_spmd

    x = np.ascontiguousarray(hidden_states, dtype=np.float32)
    assert x.shape == (_B, _CH, _TI, _HI, _WI), x.shape
    had = _build_hadamard()

    nc = _get_nc()
    in_maps = [
        {"x": _pack_core(x[b], jq), "had": had}
        for b in range(_B)
        for jq in range(_JQ)
    ]
    kw = {}
    if os.environ.get("KTRACE"):
        kw = dict(trace=True, trace_cores=[0])
    res = run_bass_kernel_spmd(nc, in_maps, list(range(_N_CORES)), **kw)
    _cached["last"] = res

    out = np.empty((_B, _C_OUT, 2 * _TI - 1, 2 * _HI, 2 * _WI), dtype=np.float32)
    for ci in range(_N_CORES):
        b, jq = divmod(ci, _JQ)
        o = np.asarray(res.results[ci]["out"])
        # (jc, q, dt, dh, i, c, w) -> (c, i, dt, jc, q, dh, w)
        v = o.reshape(2, 32, 2, 2, _TI, 3, 512).transpose(5, 4, 2, 0, 1, 3, 6)
        v = v.reshape(3, 2 * _TI, 2 * _HJ, 512)
        out[b, :, :, jq * 2 * _HJ : (jq + 1) * 2 * _HJ, :] = v[:, 1:, :, :]
    return out


# revision 12
# speedup vs baseline: 2.8251x; 2.8251x over previous
"""
CosmosUnpatcher3d (inverse 3D Haar wavelet, PATCH_SIZE=2) on 8 Trainium2
NeuronCores.

Math: input  x[b, ch, i, j, k] with ch = 3*g + c, g = (bt, bh, bw) bits
      output y[b, c, t, h, w]  with t = 2i+dt, h = 2j+dh, w = 2k+dw
      y = sum_g (-1)^(bt*dt + bh*dh + bw*dw) * x[...]
(the Haar taps (1/sqrt2)^3 times the final sqrt(8) rescale cancel to
exactly 1.0), then the t=0 plane is dropped (17 output t-planes).

This is an 8-point Hadamard transform across the 8 subband planes,
done as a 3-stage butterfly. On this backend per-instruction overhead
dominates, so the kernel is built from the fewest, largest possible
instructions: each butterfly stage is exactly 2 DVE tensor_tensor ops
(one add, one subtract) over multi-plane strided views.

Sharding: 8 cores = batch(2) x H-quarters(4). Each core processes its
(24, 9, 64, 256) shard in 2 rounds (H-halves of 32 rows):
  round = [128 partitions, 8 planes x 1728] resident in SBUF
  in-DMA (7.08 MB) -> 6 tensor_tensor ops -> out-DMA (7.08 MB)
Host packs shards partition-major (pure data movement; all arithmetic
happens on device) and scatters the 8 result planes into the strided
output positions.
"""

import numpy as np

_N_CORES = 8
_B, _CH, _TI, _HI, _WI = 2, 24, 9, 256, 256
_C_OUT = 3
_JQ = 4               # H-quarter cores per batch entry
_HJ = _HI // _JQ      # 64 input rows per core
_PL = 1728            # per-plane elems per partition (3*9*32*256 / 128)
_F = 8 * _PL          # free-dim elems per partition per round

_cached = {}


def _build_nc(repeat=1):
    import concourse.bacc as bacc
    import concourse.mybir as mybir
    from concourse.tile import TileContext
    from concourse.mybir import AluOpType
    from contextlib import ExitStack

    f32 = mybir.dt.float32
    add, sub = AluOpType.add, AluOpType.subtract
    nc = bacc.Bacc()
    X = nc.declare_dram_parameter("x", [2, 128, _F], f32, isOutput=False)
    O = nc.declare_dram_parameter("out", [2, 128, _F], f32, isOutput=True)

    H = _F // 2   # 6912
    Q = _F // 4   # 3456

    with TileContext(nc) as tc, ExitStack() as ctx:
        pa = ctx.enter_context(tc.tile_pool(name="pa", bufs=1))
        pb = ctx.enter_context(tc.tile_pool(name="pb", bufs=1))
        pc = ctx.enter_context(tc.tile_pool(name="pc", bufs=1))

        for _rep in range(repeat):
            for jc in range(2):
                t0 = pa.tile([128, _F], f32, tag="a")
                nc.sync.dma_start(out=t0[:], in_=X[jc])
                s1 = pb.tile([128, _F], f32, tag="b")
                # stage 1 (bt -> dt): planes {0..3} vs {4..7}
                nc.vector.tensor_tensor(s1[:, 0:H], t0[:, 0:H], t0[:, H:_F], add)
                nc.vector.tensor_tensor(s1[:, H:_F], t0[:, 0:H], t0[:, H:_F], sub)
                # stage 2 (bh -> dh): within each dt half, {0,1} vs {2,3}
                s2 = pa.tile([128, _F], f32, tag="a")  # reuses t0's slot
                s1v = s1[:].rearrange("p (bt x) -> p bt x", bt=2)
                s2v = s2[:].rearrange("p (bt x) -> p bt x", bt=2)
                nc.vector.tensor_tensor(
                    s2v[:, :, 0:Q], s1v[:, :, 0:Q], s1v[:, :, Q:H], add
                )
                nc.vector.tensor_tensor(
                    s2v[:, :, Q:H], s1v[:, :, 0:Q], s1v[:, :, Q:H], sub
                )
                # stage 3 (bw -> dw): within each (dt,dh) pair, even vs odd
                z = pc.tile([128, _F], f32, tag="c")
                s2q = s2[:].rearrange("p (q x) -> p q x", q=4)
                zq = z[:].rearrange("p (q x) -> p q x", q=4)
                nc.vector.tensor_tensor(
                    zq[:, :, 0:_PL], s2q[:, :, 0:_PL], s2q[:, :, _PL : 2 * _PL], add
                )
                nc.vector.tensor_tensor(
                    zq[:, :, _PL : 2 * _PL],
                    s2q[:, :, 0:_PL],
                    s2q[:, :, _PL : 2 * _PL],
                    sub,
                )
                nc.scalar.dma_start(out=O[jc], in_=z[:])
    nc.finalize()
    return nc


def _get_nc():
    import os

    rep = int(os.environ.get("K_NC_REPEAT", "1"))
    key = ("nc", rep)
    if key not in _cached:
        _cached[key] = _build_nc(rep)
    return _cached[key]


def _pack_core(xb, jq):
    """xb: (24, 9, 256, 256) one batch entry; -> (2, 128, _F) packed."""
    xs = xb[:, :, jq * _HJ : (jq + 1) * _HJ, :]          # (24, 9, 64, 256)
    v = xs.reshape(8, 3, _TI, 2, 32, 256)                # (g, c, i, jc, jl, k)
    v = v.transpose(3, 1, 2, 4, 5, 0)                    # (jc, c, i, jl, k, g)
    v = np.ascontiguousarray(v).reshape(2, 128, _PL, 8)  # (jc, p, r, g)
    v = v.transpose(0, 1, 3, 2)                          # (jc, p, g, r)
    return np.ascontiguousarray(v).reshape(2, 128, _F)


def kernel(hidden_states: np.ndarray) -> np.ndarray:
    import os
    from concourse.bass_utils import run_bass_kernel_spmd

    x = np.ascontiguousarray(hidden_states, dtype=np.float32)
    assert x.shape == (_B, _CH, _TI, _HI, _WI), x.shape

    nc = _get_nc()
    in_maps = [
        {"x": _pack_core(x[b], jq)} for b in range(_B) for jq in range(_JQ)
    ]
    kw = {}
    if os.environ.get("KTRACE"):
        kw = dict(trace=True, trace_cores=[0])
    res = run_bass_kernel_spmd(nc, in_maps, list(range(_N_CORES)), **kw)
    _cached["last"] = res

    out = np.empty((_B, _C_OUT, 2 * _TI - 1, 2 * _HI, 2 * _WI), dtype=np.float32)
    tmp = np.empty((_C_OUT, 2 * _TI, 2 * _HJ, 2 * _WI), dtype=np.float32)
    for ci in range(_N_CORES):
        b, jq = divmod(ci, _JQ)
        o = np.asarray(res.results[ci]["out"])           # (2, 128, _F)
        y = o.reshape(2, 128, 8, _PL).transpose(0, 2, 1, 3)
        y = y.reshape(2, 8, _C_OUT, _TI, 32, 256)        # (jc, slot, c, i, jl, k)
        for jc in range(2):
            for slot in range(8):
                dt, dh, dw = (slot >> 2) & 1, (slot >> 1) & 1, slot & 1
                tmp[
                    :, dt::2, jc * 64 + dh : jc * 64 + 64 : 2, dw::2
                ] = y[jc, slot]
        out[b, :, :, jq * 2 * _HJ : (jq + 1) * 2 * _HJ, :] = tmp[:, 1:]
    return out


# revision 13
# speedup vs baseline: 5.1180x; 1.8116x over previous
"""
CosmosUnpatcher3d (inverse 3D Haar wavelet, PATCH_SIZE=2) on 8 Trainium2
NeuronCores.

Math: input  x[b, ch, i, j, k] with ch = 3*g + c, g = (bt, bh, bw) bits
      output y[b, c, t, h, w]  with t = 2i+dt, h = 2j+dh, w = 2k+dw
      y = sum_g (-1)^(bt*dt + bh*dh + bw*dw) * x[...]
(the Haar taps (1/sqrt2)^3 times the final sqrt(8) rescale cancel to
exactly 1.0), then the t=0 plane is dropped (17 output t-planes).

This is an 8-point Hadamard transform across the 8 subband planes,
done as a 3-stage butterfly. On this backend per-instruction overhead
dominates, so the kernel is built from the fewest, largest possible
instructions: each butterfly stage is exactly 2 DVE tensor_tensor ops
(one add, one subtract) over multi-plane strided views.

Sharding: 8 cores = batch(2) x H-quarters(4). Each core processes its
(24, 9, 64, 256) shard in 2 rounds (H-halves of 32 rows):
  round = [128 partitions, 8 planes x 1728] resident in SBUF
  in-DMA (7.08 MB) -> 6 tensor_tensor ops -> out-DMA (7.08 MB)
Host packs shards partition-major (pure data movement; all arithmetic
happens on device) and scatters the 8 result planes into the strided
output positions.
"""

import numpy as np

_N_CORES = 8
_B, _CH, _TI, _HI, _WI = 2, 24, 9, 256, 256
_C_OUT = 3
_JQ = 4               # H-quarter cores per batch entry
_HJ = _HI // _JQ      # 64 input rows per core
_PL = 1728            # per-plane elems per partition (3*9*32*256 / 128)
_F = 8 * _PL          # free-dim elems per partition per round

_cached = {}


def _build_nc(repeat=1):
    import concourse.bacc as bacc
    import concourse.mybir as mybir
    from concourse.tile import TileContext
    from concourse.mybir import AluOpType
    from contextlib import ExitStack

    f32 = mybir.dt.float32
    add, sub = AluOpType.add, AluOpType.subtract
    nc = bacc.Bacc()
    X = nc.declare_dram_parameter("x", [2, 128, _F], f32, isOutput=False)
    O = nc.declare_dram_parameter("out", [2, 128, _F], f32, isOutput=True)

    H = _F // 2   # 6912
    Q = _F // 4   # 3456

    with TileContext(nc) as tc, ExitStack() as ctx:
        pa = ctx.enter_context(tc.tile_pool(name="pa", bufs=1))
        pb = ctx.enter_context(tc.tile_pool(name="pb", bufs=1))
        pc = ctx.enter_context(tc.tile_pool(name="pc", bufs=1))

        for _rep in range(repeat):
            for jc in range(2):
                t0 = pa.tile([128, _F], f32, tag="a")
                nc.sync.dma_start(out=t0[:], in_=X[jc])
                s1 = pb.tile([128, _F], f32, tag="b")
                # stage 1 (bt -> dt): planes {0..3} vs {4..7} — flat
                nc.vector.tensor_tensor(s1[:, 0:H], t0[:, 0:H], t0[:, H:_F], add)
                nc.vector.tensor_tensor(s1[:, H:_F], t0[:, 0:H], t0[:, H:_F], sub)
                # stage 2 (bh -> dh): within each dt half, {0,1} vs {2,3}
                s2 = pa.tile([128, _F], f32, tag="a")  # reuses t0's slot
                for dt in range(2):
                    b0 = dt * H
                    nc.vector.tensor_tensor(
                        s2[:, b0 : b0 + Q], s1[:, b0 : b0 + Q],
                        s1[:, b0 + Q : b0 + H], add,
                    )
                    nc.vector.tensor_tensor(
                        s2[:, b0 + Q : b0 + H], s1[:, b0 : b0 + Q],
                        s1[:, b0 + Q : b0 + H], sub,
                    )
                # stage 3 (bw -> dw): within each (dt,dh) pair, even vs odd
                z = pc.tile([128, _F], f32, tag="c")
                for qb in range(4):
                    b0 = qb * Q
                    nc.vector.tensor_tensor(
                        z[:, b0 : b0 + _PL], s2[:, b0 : b0 + _PL],
                        s2[:, b0 + _PL : b0 + Q], add,
                    )
                    nc.vector.tensor_tensor(
                        z[:, b0 + _PL : b0 + Q], s2[:, b0 : b0 + _PL],
                        s2[:, b0 + _PL : b0 + Q], sub,
                    )
                nc.scalar.dma_start(out=O[jc], in_=z[:])
    nc.finalize()
    return nc


def _get_nc():
    import os

    rep = int(os.environ.get("K_NC_REPEAT", "1"))
    key = ("nc", rep)
    if key not in _cached:
        _cached[key] = _build_nc(rep)
    return _cached[key]


def _pack_core(xb, jq):
    """xb: (24, 9, 256, 256) one batch entry; -> (2, 128, _F) packed."""
    xs = xb[:, :, jq * _HJ : (jq + 1) * _HJ, :]          # (24, 9, 64, 256)
    v = xs.reshape(8, 3, _TI, 2, 32, 256)                # (g, c, i, jc, jl, k)
    v = v.transpose(3, 1, 2, 4, 5, 0)                    # (jc, c, i, jl, k, g)
    v = np.ascontiguousarray(v).reshape(2, 128, _PL, 8)  # (jc, p, r, g)
    v = v.transpose(0, 1, 3, 2)                          # (jc, p, g, r)
    return np.ascontiguousarray(v).reshape(2, 128, _F)


def kernel(hidden_states: np.ndarray) -> np.ndarray:
    import os
    from concourse.bass_utils import run_bass_kernel_spmd

    x = np.ascontiguousarray(hidden_states, dtype=np.float32)
    assert x.shape == (_B, _CH, _TI, _HI, _WI), x.shape

    nc = _get_nc()
    in_maps = [
        {"x": _pack_core(x[b], jq)} for b in range(_B) for jq in range(_JQ)
    ]
    kw = {}
    if os.environ.get("KTRACE"):
        kw = dict(trace=True, trace_cores=[0])
    res = run_bass_kernel_spmd(nc, in_maps, list(range(_N_CORES)), **kw)
    _cached["last"] = res

    out = np.empty((_B, _C_OUT, 2 * _TI - 1, 2 * _HI, 2 * _WI), dtype=np.float32)
    tmp = np.empty((_C_OUT, 2 * _TI, 2 * _HJ, 2 * _WI), dtype=np.float32)
    for ci in range(_N_CORES):
        b, jq = divmod(ci, _JQ)
        o = np.asarray(res.results[ci]["out"])           # (2, 128, _F)
        y = o.reshape(2, 128, 8, _PL).transpose(0, 2, 1, 3)
        y = y.reshape(2, 8, _C_OUT, _TI, 32, 256)        # (jc, slot, c, i, jl, k)
        for jc in range(2):
            for slot in range(8):
                dt, dh, dw = (slot >> 2) & 1, (slot >> 1) & 1, slot & 1
                tmp[
                    :, dt::2, jc * 64 + dh : jc * 64 + 64 : 2, dw::2
                ] = y[jc, slot]
        out[b, :, :, jq * 2 * _HJ : (jq + 1) * 2 * _HJ, :] = tmp[:, 1:]
    return out
